# revision 2
# baseline (speedup 1.0000x reference)
"""Viterbi CRF decode on Trainium2 (Bass), 8-core data-parallel. v2.

Same layout/phases as the v1 baseline (see its docstring). v2 moves the
bp-encode multiply (W = EB * CT2) to the Pool (gpsimd) engine, running
2 steps behind the DVE scan:

  - RB becomes a 4-slot ring: slot t%4 = [V_t (128) | W_{t-2} (128)].
    The dual 8-segment reduce at step t emits [part_t | bpw_{t-2}] into
    PHBW[8t:8t+8]; bp row k = bpw_{k+1} now sits at cols 8k+28.
  - DVE computes eq_t (is_equal V_t vs part_t broadcast -> EB ring of 2)
    and signals p_sem; Pool waits p_sem, multiplies EB*CT2 into slot
    (t+2)%4's W half, signals w_sem. DVE waits w_sem before RED2_t
    (t >= 3) and before the two tail W-only flush reduces.
  - Semaphore thresholds are per-engine counter REGISTERS (reg_add +1
    per wait/inc point), so the hardware reps loop needs no sem resets:
    counters continue monotonically across reps.
  - eq_t doubles as the spacer between RED2_t's PHBW write and
    p4build_t's stream-shuffle read; the old mult slot between
    p4build_t and add_{t+1} becomes the chunk build (1/64 steps) or a
    cheap 8-element copy.

Everything else (fp association (feats+trans)+part via SCH chunks,
encode 31-i, pointer phase, backward chase) is identical to v1, so the
decode is bit-exact vs the jax reference.

reps is a HARDWARE loop: program size independent of reps.
"""

import numpy as np
from contextlib import ExitStack

import concourse.bass as bass
import concourse.mybir as mybir
from concourse.bass_utils import run_bass_kernel_spmd

F32 = mybir.dt.float32
I32 = mybir.dt.int32
AX = mybir.AxisListType
OP = mybir.AluOpType

T = 32
START = 30
END = 31
NCORES = 8


def build_nc(S, reps=1, sim_compat=False):
    nc = bass.Bass(detect_race_conditions=False)
    ft_d = nc.declare_dram_parameter("ft", [128, 4 * S], F32, isOutput=False)
    mkf_d = nc.declare_dram_parameter("mkf", [128, 4 * S + 4], F32, isOutput=False)
    tt_d = nc.declare_dram_parameter("tt", [128, 32], F32, isOutput=False)
    cst_d = nc.declare_dram_parameter("cst", [128, 64], F32, isOutput=False)
    ct2_d = nc.declare_dram_parameter("ct2", [128, 128], F32, isOutput=False)
    dec_d = nc.declare_dram_parameter("dec", [128, S], I32, isOutput=True)

    K = S - 1  # bp rows k in [0, K)

    with ExitStack() as ctx:
        def sb(name, shape, dt=F32):
            return ctx.enter_context(nc.sbuf_tensor(name, shape, dt))

        FT = sb("FT", [128, 4 * S])
        MKF = sb("MKF", [128, 4 * S + 4])
        TT = sb("TT", [128, 32])
        # interleaved history: [part_t (4) | bpw_{t-2} (4)] at cols 8t
        PHBW = sb("PHBW", [128, 8 * S + 24])
        RB = sb("RB", [128, 4 * 256])  # 4 slots x [V | W]
        EB = sb("EB", [128, 2 * 128])  # 2 slots
        XS = sb("XS", [128, 4 * S + 32])
        XS2 = sb("XS2", [128, 4 * S + 32])
        SCR = sb("SCR", [128, 4 * K])
        ALF = sb("ALF", [128, 4 * S])
        ALB = sb("ALB", [128, 4 * S])
        SCH = sb("SCH", [128, 64 * 128])
        CT2 = sb("CT2", [128, 128])
        P4 = sb("P4", [128, 128])
        T32 = sb("T32", [128, 32])
        DEC = sb("DEC", [128, S])
        DECI = sb("DECI", [128, S], I32)
        CST = sb("CST", [128, 64])
        TB32 = sb("TB32", [128, 1024])  # 32-slot ring of transposed bp rows
        TEND = sb("TEND", [128, 32])
        LPP = sb("LPP", [128, 32])
        TLP = sb("TLP", [128, 32])
        CAND = sb("CAND", [128, 32])
        MX = sb("MX", [128, 1])
        EQC = sb("EQC", [128, 32])
        PW = sb("PW", [128, 1])
        P32 = sb("P32", [128, 32])
        PR = sb("PR", [128, 32])
        SC = sb("SC", [128, 32])

        def rbV(t):
            return RB[:, 256 * (t % 4):256 * (t % 4) + 128]

        def rbW(t):
            # W_t -> slot (t+2)%4's W half, consumed by RED2_{t+2}
            s = (t + 2) % 4
            return RB[:, 256 * s + 128:256 * s + 256]

        def rbFull(t):
            return RB[:, 256 * (t % 4):256 * (t % 4) + 256]

        def ebS(t):
            return EB[:, 128 * (t % 2):128 * (t % 2) + 128]

        with (
            nc.semaphore() as dma_sem,
            nc.semaphore() as done_sem,
            nc.semaphore() as p_sem,
            nc.semaphore() as w_sem,
            nc.Block() as block,
        ):
            @block.sync
            def _(sync):
                sync.dma_start(out=FT[:], in_=ft_d[:]).then_inc(dma_sem, 16)
                sync.dma_start(out=MKF[:], in_=mkf_d[:]).then_inc(dma_sem, 16)
                sync.dma_start(out=TT[:], in_=tt_d[:]).then_inc(dma_sem, 16)
                sync.dma_start(out=CST[:], in_=cst_d[:]).then_inc(dma_sem, 16)
                sync.dma_start(out=CT2[:], in_=ct2_d[:]).then_inc(dma_sem, 16)
                sync.wait_ge(done_sem, 1)
                sync.dma_start(out=dec_d[:], in_=DECI[:]).then_inc(dma_sem, 16)

            # bulk score views: SCH[p, 128u + 32br + i] = feats[b,t0+u,j]
            # + trans[i,j], built one 64-step chunk at a time
            tt_c = TT[:].unsqueeze(1).unsqueeze(1).broadcast_to([128, 64, 4, 32])

            def sch_src(c):
                return FT[:, 256 * c:256 * (c + 1)].rearrange(
                    "p (u b) -> p u b", b=4).unsqueeze(3).broadcast_to([128, 64, 4, 32])

            p4_blk = P4[:].rearrange("p (b i) -> p b i", b=4)

            def p4_build(v, t):
                if sim_compat:
                    for br in range(4):
                        v.transpose(out=P4[:, 32 * br:32 * br + 32],
                                    in_=PHBW[:, 8 * t + br:8 * t + br + 1].broadcast_to([128, 32]))
                else:
                    v.transpose(out=p4_blk,
                                in_=PHBW[:, 8 * t:8 * t + 4].unsqueeze(2).broadcast_to([128, 4, 32]))

            def emit_body(v, wth):
                # init t=0: part0[b, j] = feats[b,0,j] + trans[START, j]
                v.tensor_scalar_add(out=PHBW[:, 0:4], in0=FT[:, 0:4],
                                    scalar1=TT[:, START:START + 1])
                v.tensor_copy(out=CAND[:], in_=TEND[:])  # spacer
                v.tensor_tensor(out=SCH[:].rearrange("p (u b i) -> p u b i", b=4, i=32),
                                in0=sch_src(0), in1=tt_c, op=OP.add)
                p4_build(v, 0)
                v.tensor_copy(out=SC[:], in_=CST[:, 32:64])  # spacer
                # V_1 = scores_1 + P4_0
                v.tensor_tensor(out=rbV(1), in0=SCH[:, 128:256],
                                in1=P4[:], op=OP.add)

                # ---- forward scan ----
                for t in range(1, S - 1):
                    if t >= 3:
                        v.wait_ge(w_sem, wth)
                        v.reg_add(wth, wth, 1)
                    v.tensor_reduce(out=PHBW[:, 8 * t:8 * t + 8],
                                    in_=rbFull(t).rearrange("p (s i) -> p s i", s=8),
                                    axis=AX.X, op=OP.max)
                    v.tensor_tensor(
                        out=ebS(t).rearrange("p (b i) -> p b i", b=4),
                        in0=rbV(t).rearrange("p (b i) -> p b i", b=4),
                        in1=PHBW[:, 8 * t:8 * t + 4].unsqueeze(2).broadcast_to([128, 4, 32]),
                        op=OP.is_equal).then_inc(p_sem, 1)
                    p4_build(v, t)
                    u1 = (t + 1) % 64
                    c1 = (t + 1) // 64
                    if u1 == 0:
                        v.tensor_tensor(
                            out=SCH[:].rearrange("p (u b i) -> p u b i", b=4, i=32),
                            in0=sch_src(c1), in1=tt_c, op=OP.add)
                    else:
                        v.tensor_copy(out=CAND[:, 0:8], in_=CST[:, 0:8])  # spacer
                    v.tensor_tensor(out=rbV(t + 1),
                                    in0=SCH[:, 128 * u1:128 * u1 + 128],
                                    in1=P4[:], op=OP.add)

                # tail t = S-1: last partition + bpw_{S-3}; flush bpw_{S-2}, bpw_{S-1}
                tl = S - 1
                v.wait_ge(w_sem, wth)
                v.reg_add(wth, wth, 1)
                v.tensor_reduce(out=PHBW[:, 8 * tl:8 * tl + 8],
                                in_=rbFull(tl).rearrange("p (s i) -> p s i", s=8),
                                axis=AX.X, op=OP.max)
                v.tensor_tensor(
                    out=ebS(tl).rearrange("p (b i) -> p b i", b=4),
                    in0=rbV(tl).rearrange("p (b i) -> p b i", b=4),
                    in1=PHBW[:, 8 * tl:8 * tl + 4].unsqueeze(2).broadcast_to([128, 4, 32]),
                    op=OP.is_equal).then_inc(p_sem, 1)
                v.wait_ge(w_sem, wth)
                v.reg_add(wth, wth, 1)
                v.tensor_reduce(out=PHBW[:, 8 * S + 4:8 * S + 8],
                                in_=RB[:, 256 * (S % 4) + 128:256 * (S % 4) + 256]
                                .rearrange("p (b i) -> p b i", b=4),
                                axis=AX.X, op=OP.max)
                v.wait_ge(w_sem, wth)
                v.reg_add(wth, wth, 1)
                v.tensor_reduce(out=PHBW[:, 8 * S + 12:8 * S + 16],
                                in_=RB[:, 256 * ((S + 1) % 4) + 128:256 * ((S + 1) % 4) + 256]
                                .rearrange("p (b i) -> p b i", b=4),
                                axis=AX.X, op=OP.max)

                # ---- last_partition by-i-partition: max over t of PH + ALB ----
                ph_tb = PHBW[:, 0:8 * S].rearrange("p (t c) -> p t c", c=8)[:, :, 0:4]
                alb_tb = ALB[:].rearrange("p (t b) -> p t b", b=4)
                xs_tb = XS[:, 0:4 * S].rearrange("p (t b) -> p t b", b=4)
                v.tensor_tensor(out=xs_tb, in0=ph_tb, in1=alb_tb, op=OP.add)
                v.tensor_reduce(out=LPP[:, 0:4],
                                in_=XS[:, 0:4 * S].rearrange("p (t b) -> p b t", b=4),
                                axis=AX.X, op=OP.max)

                # bp decode + mask: bp row k = bpw_{k+1} at PHBW cols 8k+28
                bp_src = PHBW[:, 28:28 + 8 * K].rearrange("p (k c) -> p k c", c=8)[:, :, 0:4]
                xs2_kb = XS2[:, 0:4 * K].rearrange("p (k b) -> p k b", b=4)
                scr_kb = SCR[:].rearrange("p (k b) -> p k b", b=4)
                mkf_kb = MKF[:, 4:4 * K + 4].rearrange("p (k b) -> p k b", b=4)
                v.tensor_scalar(out=xs2_kb, in0=bp_src,
                                scalar1=-1.0, scalar2=31.0, op0=OP.mult, op1=OP.add)

                # pointer = argmax_i(LP[b,i] + trans[i,END]); one-time tail.
                v.transpose(out=TLP[:], in_=LPP[:])
                v.tensor_tensor(out=scr_kb, in0=xs2_kb, in1=mkf_kb, op=OP.mult)
                v.tensor_tensor(out=CAND[:], in0=TLP[:], in1=TEND[:], op=OP.add)
                v.tensor_reduce(out=MX[:], in_=CAND[:], axis=AX.X, op=OP.max)
                v.tensor_copy(out=PR[:], in_=CST[:, 0:32])
                v.tensor_tensor(out=EQC[:], in0=CAND[:],
                                in1=MX[:].broadcast_to([128, 32]), op=OP.is_equal)
                v.tensor_tensor(out=SC[:], in0=EQC[:], in1=CST[:, 32:64], op=OP.mult)
                v.tensor_reduce(out=PW[:], in_=SC[:], axis=AX.X, op=OP.max)
                v.tensor_copy(out=PR[:], in_=CST[:, 0:32])
                v.tensor_scalar(out=P32[:, 0:1], in0=PW[:], scalar1=-1.0,
                                scalar2=31.0, op0=OP.mult, op1=OP.add)
                v.tensor_copy(out=CAND[:], in_=TEND[:])

                # scatter pointer at k == last_pos: bp' = bp + atlast*(ptr - bp)
                v.transpose(out=T32[:], in_=P32[:])
                v.stream_shuffle(out=PR[:], in_=T32[:], mask=[0] * 32)
                v.tensor_copy(out=CAND[:], in_=TEND[:])
                pr_b = PR[:, 0:4].unsqueeze(1).broadcast_to([128, K, 4])
                bp_v = SCR[:].rearrange("p (k b) -> p k b", b=4)
                xs_v = XS[:, 0:4 * K].rearrange("p (k b) -> p k b", b=4)
                xs2_v = XS2[:, 0:4 * K].rearrange("p (k b) -> p k b", b=4)
                alf_v = ALF[:, 0:4 * K].rearrange("p (k b) -> p k b", b=4)
                v.tensor_tensor(out=xs_v, in0=pr_b, in1=bp_v, op=OP.subtract)
                v.tensor_tensor(out=xs2_v, in0=xs_v, in1=alf_v, op=OP.mult)
                v.tensor_tensor(out=xs_v, in0=bp_v, in1=xs2_v, op=OP.add)

                # ---- backward pass ----
                v.tensor_copy(out=DEC[:, S - 1:S], in_=P32[:, 0:1])
                tb32_r = TB32[:].rearrange("p (g m c) -> p g m c", g=4, m=8, c=32)

                def bigtr(k0):
                    if sim_compat:
                        for g in range(4):
                            v.transpose(
                                out=TB32[:, 32 * (k0 % 32 + 8 * g):32 * (k0 % 32 + 8 * g) + 32],
                                in_=XS[:, 4 * k0 + 32 * g:4 * k0 + 32 * g + 32])
                    else:
                        v.transpose(out=tb32_r[:, :, k0 % 32, :],
                                    in_=XS[:, 4 * k0:4 * k0 + 128].rearrange(
                                        "p (g c) -> p g c", c=32))

                for m in range(8):
                    bigtr(480 + m)
                for k in range(S - 2, -1, -1):
                    v.scalar_tensor_tensor(out=EQC[:], in0=CST[:, 0:32],
                                           scalar=DEC[:, k + 1:k + 2],
                                           in1=TB32[:, 32 * (k % 32):32 * (k % 32) + 32],
                                           op0=OP.is_equal, op1=OP.mult,
                                           accum_out=DEC[:, k:k + 1])
                    F = (k // 32) * 32
                    if k % 32 < 8 and F >= 32:
                        bigtr(F - 32 + (k % 32))
                    else:
                        v.tensor_copy(out=CAND[:, 0:8], in_=CST[:, 0:8])

                v.tensor_copy(out=CAND[:], in_=TEND[:])
                v.tensor_copy(out=DECI[:], in_=DEC[:])

            @block.vector
            def _(v):
                v.wait_ge(dma_sem, 80)
                # hoisted rep-invariant prep
                v.stream_shuffle(out=TEND[:], in_=TT[:], mask=[END] * 32)
                v.memset(XS[:, 4 * K:], 0.0)
                for s in range(4):
                    v.memset(RB[:, 256 * s + 128:256 * s + 256], 0.0)
                v.memset(P32[:], 0.0)
                v.memset(LPP[:], 0.0)
                v.tensor_sub(out=ALF[:], in0=MKF[:, 0:4 * S], in1=MKF[:, 4:4 * S + 4])
                v.tensor_scalar(out=ALB[:], in0=ALF[:], scalar1=1.0,
                                scalar2=1e30, op0=OP.subtract, op1=OP.mult)
                wth = v.alloc_register("wth")
                v.reg_mov(wth, 1)
                with v.Fori(0, reps):
                    emit_body(v, wth)
                v.drain().then_inc(done_sem, 1)

            @block.gpsimd
            def _(g):
                g.wait_ge(dma_sem, 80)
                pth = g.alloc_register("pth")
                g.reg_mov(pth, 1)
                with g.Fori(0, reps):
                    for t in range(1, S):
                        g.wait_ge(p_sem, pth)
                        g.reg_add(pth, pth, 1)
                        g.tensor_tensor(out=rbW(t), in0=ebS(t), in1=CT2[:],
                                        op=OP.mult).then_inc(w_sem, 1)

    return nc


def pack_inputs(feats, transitions, mask, S):
    """Host-side layout packing (pure data movement)."""
    trans = np.ascontiguousarray(np.asarray(transitions, np.float32))
    ttrep = np.ascontiguousarray(np.tile(trans.T, (4, 1)))  # [128, 32]
    iota = np.arange(32, dtype=np.float32)
    cst = np.ascontiguousarray(
        np.tile(np.concatenate([iota, 31.0 - iota])[None, :], (128, 1)))
    ct2 = np.ascontiguousarray(
        np.tile(np.tile(31.0 - iota, 4)[None, :], (128, 1)).astype(np.float32))
    in_maps = []
    bc = 16
    for c in range(NCORES):
        f = np.asarray(feats[bc * c:bc * c + bc], np.float32)  # [16, S, 32]
        ft = np.ascontiguousarray(
            f.reshape(4, 4, S, T).transpose(0, 3, 2, 1).reshape(128, 4 * S))
        m = np.asarray(mask[bc * c:bc * c + bc]).astype(np.float32)  # [16, S]
        mk = np.broadcast_to(
            m.reshape(4, 1, 4, S).transpose(0, 1, 3, 2), (4, 32, S, 4))
        mk = mk.reshape(128, 4 * S)
        mkp = np.zeros((128, 4 * S + 4), np.float32)
        mkp[:, :4 * S] = mk
        in_maps.append({"ft": ft, "mkf": mkp, "tt": ttrep, "cst": cst, "ct2": ct2})
    return in_maps


def unpack_outputs(results, S):
    out = np.empty((128, S), np.int32)
    bc = 16
    for c in range(NCORES):
        d = np.asarray(results[c]["dec"]).reshape(4, 32, S)
        out[bc * c:bc * c + bc] = d[:, 0:4, :].reshape(16, S)
    return out


_NC_CACHE = {}


def kernel(feats, transitions, mask):
    B, S, Tin = feats.shape
    assert (B, Tin) == (128, 32)
    if S not in _NC_CACHE:
        _NC_CACHE[S] = build_nc(S)
    nc = _NC_CACHE[S]
    in_maps = pack_inputs(feats, transitions, mask, S)
    res = run_bass_kernel_spmd(nc, in_maps, list(range(NCORES)))
    return unpack_outputs(res.results, S)


# revision 3
# speedup vs baseline: 1.7066x; 1.7066x over previous
"""Viterbi CRF decode on Trainium2 (Bass), 8-core data-parallel. v2.

Same layout/phases as the v1 baseline (see its docstring). v2 moves the
bp-encode multiply (W = EB * CT2) to the Pool (gpsimd) engine, running
2 steps behind the DVE scan:

  - RB becomes a 4-slot ring: slot t%4 = [V_t (128) | W_{t-2} (128)].
    The dual 8-segment reduce at step t emits [part_t | bpw_{t-2}] into
    PHBW[8t:8t+8]; bp row k = bpw_{k+1} now sits at cols 8k+28.
  - DVE computes eq_t (is_equal V_t vs part_t broadcast -> EB ring of 2)
    and signals p_sem; Pool waits p_sem, multiplies EB*CT2 into slot
    (t+2)%4's W half, signals w_sem. DVE waits w_sem before RED2_t
    (t >= 3) and before the two tail W-only flush reduces.
  - Semaphore thresholds are per-engine counter REGISTERS (reg_add +1
    per wait/inc point), so the hardware reps loop needs no sem resets:
    counters continue monotonically across reps.
  - eq_t doubles as the spacer between RED2_t's PHBW write and
    p4build_t's stream-shuffle read; the old mult slot between
    p4build_t and add_{t+1} becomes the chunk build (1/64 steps) or a
    cheap 8-element copy.

Everything else (fp association (feats+trans)+part via SCH chunks,
encode 31-i, pointer phase, backward chase) is identical to v1, so the
decode is bit-exact vs the jax reference.

reps is a HARDWARE loop: program size independent of reps.
"""

import numpy as np
from contextlib import ExitStack

import concourse.bass as bass
import concourse.mybir as mybir
from concourse.bass_utils import run_bass_kernel_spmd

F32 = mybir.dt.float32
I32 = mybir.dt.int32
AX = mybir.AxisListType
OP = mybir.AluOpType

T = 32
START = 30
END = 31
NCORES = 8


def build_nc(S, reps=1, sim_compat=False):
    nc = bass.Bass(detect_race_conditions=False)
    ft_d = nc.declare_dram_parameter("ft", [128, 4 * S], F32, isOutput=False)
    mkf_d = nc.declare_dram_parameter("mkf", [128, 4 * S + 4], F32, isOutput=False)
    tt_d = nc.declare_dram_parameter("tt", [128, 32], F32, isOutput=False)
    cst_d = nc.declare_dram_parameter("cst", [128, 64], F32, isOutput=False)
    ct2_d = nc.declare_dram_parameter("ct2", [128, 128], F32, isOutput=False)
    dec_d = nc.declare_dram_parameter("dec", [128, S], I32, isOutput=True)
    scd = nc.dram_tensor("scd", [128, 8 * 8192], F32)

    K = S - 1  # bp rows k in [0, K)

    with ExitStack() as ctx:
        def sb(name, shape, dt=F32):
            return ctx.enter_context(nc.sbuf_tensor(name, shape, dt))

        FT = sb("FT", [128, 4 * S])
        MKF = sb("MKF", [128, 4 * S + 4])
        TT = sb("TT", [128, 32])
        # interleaved history: [part_t (4) | bpw_{t-2} (4)] at cols 8t
        PHBW = sb("PHBW", [128, 8 * S + 24])
        RB = sb("RB", [128, 4 * 256])  # 4 slots x [V | W]
        EB = sb("EB", [128, 2 * 128])  # 2 slots
        XS = sb("XS", [128, 4 * S + 32])
        XS2 = sb("XS2", [128, 4 * S + 32])
        SCR = sb("SCR", [128, 4 * K])
        ALF = sb("ALF", [128, 4 * S])
        ALB = sb("ALB", [128, 4 * S])
        SCH = sb("SCH", [128, 2 * 64 * 128])  # ping/pong: chunk c -> slot c%2
        CT2 = sb("CT2", [128, 128])
        P4 = sb("P4", [128, 128])
        T32 = sb("T32", [128, 32])
        DEC = sb("DEC", [128, S])
        DECI = sb("DECI", [128, S], I32)
        CST = sb("CST", [128, 64])
        TB32 = sb("TB32", [128, 1024])  # 32-slot ring of transposed bp rows
        TEND = sb("TEND", [128, 32])
        LPP = sb("LPP", [128, 32])
        TLP = sb("TLP", [128, 32])
        CAND = sb("CAND", [128, 32])
        MX = sb("MX", [128, 1])
        EQC = sb("EQC", [128, 32])
        PW = sb("PW", [128, 1])
        P32 = sb("P32", [128, 32])
        PR = sb("PR", [128, 32])
        SC = sb("SC", [128, 32])

        def rbV(t):
            return RB[:, 256 * (t % 4):256 * (t % 4) + 128]

        def rbW(t):
            # W_t -> slot (t+2)%4's W half, consumed by RED2_{t+2}
            s = (t + 2) % 4
            return RB[:, 256 * s + 128:256 * s + 256]

        def rbFull(t):
            return RB[:, 256 * (t % 4):256 * (t % 4) + 256]

        def ebS(t):
            return EB[:, 128 * (t % 2):128 * (t % 2) + 128]

        with (
            nc.semaphore() as dma_sem,
            nc.semaphore() as done_sem,
            nc.semaphore() as p_sem,
            nc.semaphore() as w_sem,
            nc.semaphore() as sch_sem,
            nc.semaphore() as stg_sem,
            nc.semaphore() as bld_sem,
            nc.Block() as block,
        ):
            @block.sync
            def _(sync):
                sync.dma_start(out=FT[:], in_=ft_d[:]).then_inc(dma_sem, 16)
                sync.dma_start(out=MKF[:], in_=mkf_d[:]).then_inc(dma_sem, 16)
                sync.dma_start(out=TT[:], in_=tt_d[:]).then_inc(dma_sem, 16)
                sync.dma_start(out=CST[:], in_=cst_d[:]).then_inc(dma_sem, 16)
                sync.dma_start(out=CT2[:], in_=ct2_d[:]).then_inc(dma_sem, 16)
                sync.wait_ge(done_sem, 1)
                sync.dma_start(out=dec_d[:], in_=DECI[:]).then_inc(dma_sem, 16)

            # bulk score views: SCH[p, 128u + 32br + i] = feats[b,t0+u,j]
            # + trans[i,j], built one 64-step chunk at a time
            tt_c = TT[:].unsqueeze(1).unsqueeze(1).broadcast_to([128, 64, 4, 32])

            def sch_src(c):
                return FT[:, 256 * c:256 * (c + 1)].rearrange(
                    "p (u b) -> p u b", b=4).unsqueeze(3).broadcast_to([128, 64, 4, 32])

            p4_blk = P4[:].rearrange("p (b i) -> p b i", b=4)

            def p4_build(v, t):
                if sim_compat:
                    for br in range(4):
                        v.transpose(out=P4[:, 32 * br:32 * br + 32],
                                    in_=PHBW[:, 8 * t + br:8 * t + br + 1].broadcast_to([128, 32]))
                else:
                    v.transpose(out=p4_blk,
                                in_=PHBW[:, 8 * t:8 * t + 4].unsqueeze(2).broadcast_to([128, 4, 32]))

            def emit_body(v, wth, sthr):
                # init t=0: part0[b, j] = feats[b,0,j] + trans[START, j]
                v.tensor_scalar_add(out=PHBW[:, 0:4], in0=FT[:, 0:4],
                                    scalar1=TT[:, START:START + 1])
                v.tensor_copy(out=CAND[:], in_=TEND[:])  # spacer
                p4_build(v, 0)
                # xs piece t=0 (doubles as spacer): XS[0:4] = part_0 + ALB
                v.tensor_tensor(out=XS[:, 0:4], in0=PHBW[:, 0:4],
                                in1=ALB[:, 0:4], op=OP.add)
                v.wait_ge(sch_sem, sthr)       # chunk 0 present in slot 0
                v.reg_add(sthr, sthr, 16)
                # V_1 = scores_1 + P4_0
                v.tensor_tensor(out=rbV(1), in0=SCH[:, 128:256],
                                in1=P4[:], op=OP.add)

                # ---- forward scan ----
                for t in range(1, S - 1):
                    if t >= 3:
                        v.wait_ge(w_sem, wth)
                        v.reg_add(wth, wth, 1)
                    v.tensor_reduce(out=PHBW[:, 8 * t:8 * t + 8],
                                    in_=rbFull(t).rearrange("p (s i) -> p s i", s=8),
                                    axis=AX.X, op=OP.max)
                    v.tensor_tensor(
                        out=ebS(t).rearrange("p (b i) -> p b i", b=4),
                        in0=rbV(t).rearrange("p (b i) -> p b i", b=4),
                        in1=PHBW[:, 8 * t:8 * t + 4].unsqueeze(2).broadcast_to([128, 4, 32]),
                        op=OP.is_equal).then_inc(p_sem, 1)
                    p4_build(v, t)
                    u1 = (t + 1) % 64
                    c1 = (t + 1) // 64
                    if u1 == 0:
                        # chunk c1 DMA'd into slot c1%2 by Pool ~63 steps ago
                        v.wait_ge(sch_sem, sthr)
                        v.reg_add(sthr, sthr, 16)
                    # xs piece t (doubles as spacer): XS[4t:4t+4] = part_t + ALB
                    v.tensor_tensor(out=XS[:, 4 * t:4 * t + 4],
                                    in0=PHBW[:, 8 * t:8 * t + 4],
                                    in1=ALB[:, 4 * t:4 * t + 4], op=OP.add)
                    v.tensor_tensor(out=rbV(t + 1),
                                    in0=SCH[:, 8192 * (c1 % 2) + 128 * u1:
                                            8192 * (c1 % 2) + 128 * u1 + 128],
                                    in1=P4[:], op=OP.add)

                # tail t = S-1: last partition + bpw_{S-3}; flush bpw_{S-2}, bpw_{S-1}
                tl = S - 1
                v.wait_ge(w_sem, wth)
                v.reg_add(wth, wth, 1)
                v.tensor_reduce(out=PHBW[:, 8 * tl:8 * tl + 8],
                                in_=rbFull(tl).rearrange("p (s i) -> p s i", s=8),
                                axis=AX.X, op=OP.max)
                v.tensor_tensor(
                    out=ebS(tl).rearrange("p (b i) -> p b i", b=4),
                    in0=rbV(tl).rearrange("p (b i) -> p b i", b=4),
                    in1=PHBW[:, 8 * tl:8 * tl + 4].unsqueeze(2).broadcast_to([128, 4, 32]),
                    op=OP.is_equal).then_inc(p_sem, 1)
                v.tensor_tensor(out=XS[:, 4 * tl:4 * tl + 4],
                                in0=PHBW[:, 8 * tl:8 * tl + 4],
                                in1=ALB[:, 4 * tl:4 * tl + 4], op=OP.add)
                v.wait_ge(w_sem, wth)
                v.reg_add(wth, wth, 1)
                v.tensor_reduce(out=PHBW[:, 8 * S + 4:8 * S + 8],
                                in_=RB[:, 256 * (S % 4) + 128:256 * (S % 4) + 256]
                                .rearrange("p (b i) -> p b i", b=4),
                                axis=AX.X, op=OP.max)
                v.wait_ge(w_sem, wth)
                v.reg_add(wth, wth, 1)
                v.tensor_reduce(out=PHBW[:, 8 * S + 12:8 * S + 16],
                                in_=RB[:, 256 * ((S + 1) % 4) + 128:256 * ((S + 1) % 4) + 256]
                                .rearrange("p (b i) -> p b i", b=4),
                                axis=AX.X, op=OP.max)

                # ---- last_partition by-i-partition: max over t of PH + ALB ----
                v.tensor_reduce(out=LPP[:, 0:4],
                                in_=XS[:, 0:4 * S].rearrange("p (t b) -> p b t", b=4),
                                axis=AX.X, op=OP.max)

                # bp decode + mask: bp row k = bpw_{k+1} at PHBW cols 8k+28
                bp_src = PHBW[:, 28:28 + 8 * K].rearrange("p (k c) -> p k c", c=8)[:, :, 0:4]
                xs2_kb = XS2[:, 0:4 * K].rearrange("p (k b) -> p k b", b=4)
                scr_kb = SCR[:].rearrange("p (k b) -> p k b", b=4)
                mkf_kb = MKF[:, 4:4 * K + 4].rearrange("p (k b) -> p k b", b=4)
                v.tensor_scalar(out=xs2_kb, in0=bp_src,
                                scalar1=-1.0, scalar2=31.0, op0=OP.mult, op1=OP.add)

                # pointer = argmax_i(LP[b,i] + trans[i,END]); one-time tail.
                v.transpose(out=TLP[:], in_=LPP[:])
                v.tensor_tensor(out=scr_kb, in0=xs2_kb, in1=mkf_kb, op=OP.mult)
                v.tensor_tensor(out=CAND[:], in0=TLP[:], in1=TEND[:], op=OP.add)
                v.tensor_reduce(out=MX[:], in_=CAND[:], axis=AX.X, op=OP.max)
                v.tensor_copy(out=PR[:], in_=CST[:, 0:32])
                v.tensor_tensor(out=EQC[:], in0=CAND[:],
                                in1=MX[:].broadcast_to([128, 32]), op=OP.is_equal)
                v.tensor_tensor(out=SC[:], in0=EQC[:], in1=CST[:, 32:64], op=OP.mult)
                v.tensor_reduce(out=PW[:], in_=SC[:], axis=AX.X, op=OP.max)
                v.tensor_copy(out=PR[:], in_=CST[:, 0:32])
                v.tensor_scalar(out=P32[:, 0:1], in0=PW[:], scalar1=-1.0,
                                scalar2=31.0, op0=OP.mult, op1=OP.add)
                v.tensor_copy(out=CAND[:], in_=TEND[:])

                # scatter pointer at k == last_pos: bp' = bp + atlast*(ptr - bp)
                v.transpose(out=T32[:], in_=P32[:])
                v.stream_shuffle(out=PR[:], in_=T32[:], mask=[0] * 32)
                v.tensor_copy(out=CAND[:], in_=TEND[:])
                pr_b = PR[:, 0:4].unsqueeze(1).broadcast_to([128, K, 4])
                bp_v = SCR[:].rearrange("p (k b) -> p k b", b=4)
                xs_v = XS[:, 0:4 * K].rearrange("p (k b) -> p k b", b=4)
                xs2_v = XS2[:, 0:4 * K].rearrange("p (k b) -> p k b", b=4)
                alf_v = ALF[:, 0:4 * K].rearrange("p (k b) -> p k b", b=4)
                v.tensor_tensor(out=xs_v, in0=pr_b, in1=bp_v, op=OP.subtract)
                v.tensor_tensor(out=xs2_v, in0=xs_v, in1=alf_v, op=OP.mult)
                v.tensor_tensor(out=xs_v, in0=bp_v, in1=xs2_v, op=OP.add)

                # ---- backward pass ----
                v.tensor_copy(out=DEC[:, S - 1:S], in_=P32[:, 0:1])
                tb32_r = TB32[:].rearrange("p (g m c) -> p g m c", g=4, m=8, c=32)

                def bigtr(k0):
                    if sim_compat:
                        for g in range(4):
                            v.transpose(
                                out=TB32[:, 32 * (k0 % 32 + 8 * g):32 * (k0 % 32 + 8 * g) + 32],
                                in_=XS[:, 4 * k0 + 32 * g:4 * k0 + 32 * g + 32])
                    else:
                        v.transpose(out=tb32_r[:, :, k0 % 32, :],
                                    in_=XS[:, 4 * k0:4 * k0 + 128].rearrange(
                                        "p (g c) -> p g c", c=32))

                for m in range(8):
                    bigtr(480 + m)
                for k in range(S - 2, -1, -1):
                    v.scalar_tensor_tensor(out=EQC[:], in0=CST[:, 0:32],
                                           scalar=DEC[:, k + 1:k + 2],
                                           in1=TB32[:, 32 * (k % 32):32 * (k % 32) + 32],
                                           op0=OP.is_equal, op1=OP.mult,
                                           accum_out=DEC[:, k:k + 1])
                    F = (k // 32) * 32
                    if k % 32 < 8 and F >= 32:
                        bigtr(F - 32 + (k % 32))
                    else:
                        v.tensor_copy(out=CAND[:, 0:8], in_=CST[:, 0:8])

                v.tensor_copy(out=CAND[:], in_=TEND[:])
                v.tensor_copy(out=DECI[:], in_=DEC[:])

            @block.vector
            def _(v):
                v.wait_ge(dma_sem, 80)
                # hoisted rep-invariant prep
                v.stream_shuffle(out=TEND[:], in_=TT[:], mask=[END] * 32)
                v.memset(XS[:, 4 * K:], 0.0)
                for s in range(4):
                    v.memset(RB[:, 256 * s + 128:256 * s + 256], 0.0)
                v.memset(P32[:], 0.0)
                v.memset(LPP[:], 0.0)
                v.tensor_sub(out=ALF[:], in0=MKF[:, 0:4 * S], in1=MKF[:, 4:4 * S + 4])
                v.tensor_scalar(out=ALB[:], in0=ALF[:], scalar1=1.0,
                                scalar2=1e30, op0=OP.subtract, op1=OP.mult)
                # stage all 8 score chunks to DRAM scratch (one-time):
                # chunks 1..7 via slot-0 staging (ACT engine DMAs them out),
                # then chunk 0 built last and left in slot 0 for rep 0.
                for idx, c in enumerate((1, 2, 3, 4, 5, 6, 7, 0)):
                    if idx > 0:
                        v.wait_ge(stg_sem, 16 * idx)  # prev DMA-out done
                    v.tensor_tensor(
                        out=SCH[:, 0:8192].rearrange("p (u b i) -> p u b i", b=4, i=32),
                        in0=sch_src(c), in1=tt_c, op=OP.add).then_inc(bld_sem, 1)
                v.wait_ge(stg_sem, 16 * 8)
                wth = v.alloc_register("wth")
                v.reg_mov(wth, 1)
                # rep-0 chunk-0 threshold is 0 (chunk 0 left in slot 0 by
                # the staging loop); SWDGE sems must start at 0.
                sthr = v.alloc_register("sthr")
                v.reg_mov(sthr, 0)
                with v.Fori(0, reps):
                    emit_body(v, wth, sthr)
                v.drain().then_inc(done_sem, 1)

            @block.scalar
            def _(a):
                for idx, c in enumerate((1, 2, 3, 4, 5, 6, 7, 0)):
                    a.wait_ge(bld_sem, idx + 1)
                    a.dma_start(out=scd[:, 8192 * c:8192 * (c + 1)],
                                in_=SCH[:, 0:8192]).then_inc(stg_sem, 16)
                # per-rep chunk reloads: chunk c -> slot c%2 once DVE's eq
                # count shows the slot's previous chunk fully consumed.
                ath = a.alloc_register("ath")
                a.reg_mov(ath, 1)
                with a.Fori(0, reps):
                    for i, c in enumerate((1, 2, 3, 4, 5, 6, 7, 0)):
                        a.wait_ge(p_sem, ath)
                        a.reg_add(ath, ath, 64 if i < 7 else 63)
                        a.dma_start(
                            out=SCH[:, 8192 * (c % 2):8192 * (c % 2) + 8192],
                            in_=scd[:, 8192 * c:8192 * (c + 1)]
                        ).then_inc(sch_sem, 16)

            @block.gpsimd
            def _(g):
                g.wait_ge(dma_sem, 80)
                pth = g.alloc_register("pth")
                g.reg_mov(pth, 1)
                with g.Fori(0, reps):
                    for t in range(1, S):
                        g.wait_ge(p_sem, pth)
                        g.reg_add(pth, pth, 1)
                        g.tensor_tensor(out=rbW(t), in0=ebS(t), in1=CT2[:],
                                        op=OP.mult).then_inc(w_sem, 1)

    return nc


def pack_inputs(feats, transitions, mask, S):
    """Host-side layout packing (pure data movement)."""
    trans = np.ascontiguousarray(np.asarray(transitions, np.float32))
    ttrep = np.ascontiguousarray(np.tile(trans.T, (4, 1)))  # [128, 32]
    iota = np.arange(32, dtype=np.float32)
    cst = np.ascontiguousarray(
        np.tile(np.concatenate([iota, 31.0 - iota])[None, :], (128, 1)))
    ct2 = np.ascontiguousarray(
        np.tile(np.tile(31.0 - iota, 4)[None, :], (128, 1)).astype(np.float32))
    in_maps = []
    bc = 16
    for c in range(NCORES):
        f = np.asarray(feats[bc * c:bc * c + bc], np.float32)  # [16, S, 32]
        ft = np.ascontiguousarray(
            f.reshape(4, 4, S, T).transpose(0, 3, 2, 1).reshape(128, 4 * S))
        m = np.asarray(mask[bc * c:bc * c + bc]).astype(np.float32)  # [16, S]
        mk = np.broadcast_to(
            m.reshape(4, 1, 4, S).transpose(0, 1, 3, 2), (4, 32, S, 4))
        mk = mk.reshape(128, 4 * S)
        mkp = np.zeros((128, 4 * S + 4), np.float32)
        mkp[:, :4 * S] = mk
        in_maps.append({"ft": ft, "mkf": mkp, "tt": ttrep, "cst": cst, "ct2": ct2})
    return in_maps


def unpack_outputs(results, S):
    out = np.empty((128, S), np.int32)
    bc = 16
    for c in range(NCORES):
        d = np.asarray(results[c]["dec"]).reshape(4, 32, S)
        out[bc * c:bc * c + bc] = d[:, 0:4, :].reshape(16, S)
    return out


_NC_CACHE = {}


def kernel(feats, transitions, mask):
    B, S, Tin = feats.shape
    assert (B, Tin) == (128, 32)
    if S not in _NC_CACHE:
        _NC_CACHE[S] = build_nc(S)
    nc = _NC_CACHE[S]
    in_maps = pack_inputs(feats, transitions, mask, S)
    res = run_bass_kernel_spmd(nc, in_maps, list(range(NCORES)))
    return unpack_outputs(res.results, S)
